# revision 1
# baseline (speedup 1.0000x reference)
"""Bilateral filter (5x5, sigma_space = sigma_density = 1.1) on 8 TRN2 NeuronCores.

Contract: kernel(x, gw) takes FULL inputs
    x : [4, 3, 512, 512] float32
    gw: [5, 5] float32 (normalized spatial gaussian)
returns FULL output [4, 3, 512, 512] float32.

Sharding: pure data parallel over H. Core k owns output rows [64k, 64k+64)
of every (b, c) channel; the host hands it an edge-padded strip, so the
device kernel needs no boundary handling and no inter-core communication.

Device algorithm: Taylor/separable-convolution reformulation.
With inv = 1/sigma^2 and f(u) = exp(-u^2 * inv / 2):
    exp(-(p-c)^2*inv/2) = f(p) * f(c) * exp(p*c*inv)
                        ~ f(p) * f(c) * sum_{m<=M} (inv^m/m!) p^m c^m
so (f(c) cancels in the num/den ratio, and gw = gwy x gwx is separable):
    out = num/den,  den = sum_m CP_m . CONV2[G_m],  num = sum_m CP_m . CONV2[G_{m+1}]
where G_m = f(x) * x^m (a per-pixel field), CP_m = (inv^m/m!) c^m, and
CONV2 is the separable 5x5 spatial gaussian. M=3 -> 5 fields, truncation
error ~6e-4 relative.

Layout: W(columns) on SBUF partitions; free dim is [row][channel] so every
H-direction row shift lands 4B-aligned (keeps the DVE fp16 2x/4x modes).
The fields G_m and coefficients CP_m are precomputed on the host (cheap
elementwise prep, like the padding/transposes). On device: the W-direction
conv is a banded-matrix matmul on the otherwise idle TensorEngine (fp32
PSUM accumulation); the H-direction conv is 4 packed DVE adds
(symmetric-kernel pairing, uniform scale steps on the ScalarEngine) over
all 5 fields at once; the num/den polynomial series is evaluated with both
chains packed per DVE op. All elementwise work in fp16 (DVE 2x/4x modes).
"""

import numpy as np

import concourse.bass as bass
import concourse.bacc as bacc
import concourse.tile as tile
from concourse import mybir
from concourse.bass_utils import run_bass_kernel_spmd

# ---- problem constants (hardcoded per contract) ----
B, C, H, W = 4, 3, 512, 512
K = 5
PAD = 2
SIGMA = 0.3 * ((K - 1) * 0.5 - 1) + 0.8  # 1.1
NCORES = 8
CH = B * C                    # 12 channels
RPC = H // NCORES             # 64 output rows per core
SR = RPC + 2 * PAD            # 68 input rows per channel strip
P = 128
NG = W // P                   # 4 column groups
FI = SR * CH                  # 816 free elems of input-row fields [row][ch]
FO = RPC * CH                 # 768 free elems of output-row tensors [row][ch]
M = 3                         # Taylor order: fields G_0..G_{M+1}
NF = M + 2                    # 5 fields

FP32 = mybir.dt.float32
FP16 = mybir.dt.float16
AL = mybir.AluOpType
AF = mybir.ActivationFunctionType


def _build_nc(gw: np.ndarray) -> bass.Bass:
    gw64 = np.asarray(gw, np.float64)
    gwy = gw64.sum(axis=1)            # H-direction 1D kernel (shift i)
    ky0, ky1, ky2 = float(gwy[0]), float(gwy[1]), float(gwy[2])
    # H-conv with ky2 deferred (uniform scale cancels in num/den):
    #   S' = p2*ky0/ky2 + p1*ky1/ky2 + center

    nc = bacc.Bacc(None)
    gfd = nc.declare_dram_parameter("gf", [NG, P, NF * FI], FP16,
                                    isOutput=False)
    ged = nc.declare_dram_parameter("ge", [4, NF * FI], FP16, isOutput=False)
    xcp = nc.declare_dram_parameter("xcp", [NG, P, M * 2 * FO], FP16,
                                    isOutput=False)
    b1d = nc.declare_dram_parameter("b1", [P, P], FP16, isOutput=False)
    b2d = nc.declare_dram_parameter("b2", [4, P], FP16, isOutput=False)
    out = nc.declare_dram_parameter("out", [NG, P, FO], FP32, isOutput=True)

    with tile.TileContext(nc) as tc:
        with (
            tc.tile_pool(name="const", bufs=1) as const_pool,
            tc.tile_pool(name="fields", bufs=1) as fld_pool,
            tc.tile_pool(name="ws", bufs=2) as ws_pool,
            tc.tile_pool(name="ps", bufs=4, space="PSUM") as ps_pool,
            tc.tile_pool(name="work", bufs=2) as work_pool,
            tc.tile_pool(name="res", bufs=2) as res_pool,
        ):
            b1 = const_pool.tile([P, P], FP16, tag="b1")
            nc.sync.dma_start(out=b1[:, :], in_=b1d[:, :])
            b2 = const_pool.tile([4, P], FP16, tag="b2")
            nc.sync.dma_start(out=b2[:, :], in_=b2d[:, :])

            # --- fields G_m = f(x)*x^m are precomputed on the host; each
            # group's stack (+ the 4-col tail for the edge matmul) is DMA'd
            # in whole and stays resident ---
            G = []
            for g in range(NG):
                gt = fld_pool.tile([P, NF * FI], FP16, tag=f"g{g}",
                                   name=f"gfld{g}")
                G.append(gt)
            # groups 0/1 load field-interleaved so group 0's W-conv (which
            # needs G0 and G1's edge columns) can start before the full
            # 1MB stacks land
            for m in range(NF):
                for g in (0, 1):
                    fs = slice(m * FI, (m + 1) * FI)
                    nc.sync.dma_start(out=G[g][:, fs], in_=gfd[g, :, fs])
            for g in (2, 3):
                nc.sync.dma_start(out=G[g][:, :], in_=gfd[g, :, :])
            ge = fld_pool.tile([4, NF * FI], FP16, tag="ge")
            nc.sync.dma_start(out=ge[:, :], in_=ged[:, :])

            for g in range(NG):
                # --- W-conv on TensorE: WS_m = B^T @ G_m (banded 5-tap);
                # 512+304 chunks into one 2-bank PSUM tile -> single
                # PSUM->SBUF copy per field ---
                ws = ws_pool.tile([P, NF * FI], FP16, tag="ws")
                nbr = G[g + 1] if g + 1 < NG else ge
                for m in range(NF):
                    pt = ps_pool.tile([P, 1024], FP32, tag="pt")
                    for o, sz in ((0, 512), (512, FI - 512)):
                        sl = slice(m * FI + o, m * FI + o + sz)
                        nc.tensor.matmul(pt[:, o:o + sz], b1[:, :],
                                         G[g][:, sl], start=True, stop=False)
                        nc.tensor.matmul(pt[:, o:o + sz], b2[:, :],
                                         nbr[0:4, sl], start=False, stop=True)
                    nc.scalar.activation(ws[:, m * FI:(m + 1) * FI],
                                         pt[:, 0:FI], AF.Copy)

                # --- H-conv, packed over fields x 64 rows x 12 channels ---
                def hview(t, o, f0=0, nf=NF):
                    # fields [f0:f0+nf] x rows(out) x channels, row-offset o
                    base = t[:, :]
                    return bass.AP(tensor=base.tensor,
                                   offset=base.offset + f0 * FI + o * CH,
                                   ap=[list(base.ap[0]), [FI, nf],
                                       [CH, RPC], [1, CH]])

                # S/ky2 = p2*ky0/ky2 + p1*ky1/ky2 + center. Group 0 is
                # pipeline-fill-limited: run it in field-halves with DVE
                # scale steps (no ACT round-trip); steady-state groups use
                # one packed pass with scales on the half-idle ScalarEngine.
                p2 = work_pool.tile([P, NF, RPC, CH], FP16, tag="p2")
                p1 = work_pool.tile([P, NF, RPC, CH], FP16, tag="p1")
                S = work_pool.tile([P, NF * FO], FP16, tag="S")
                Sv = S[:, :].rearrange("p (f r c) -> p f r c", f=NF, r=RPC)
                halves = ((0, 3), (3, NF)) if g == 0 else ((0, NF),)
                for f0, f1 in halves:
                    fs = slice(f0, f1)
                    nf = f1 - f0
                    nc.vector.tensor_add(p2[:, fs], hview(ws, 0, f0, nf),
                                         hview(ws, 4, f0, nf))
                    nc.vector.tensor_add(p1[:, fs], hview(ws, 1, f0, nf),
                                         hview(ws, 3, f0, nf))
                    if g == 0:
                        nc.vector.tensor_scalar_mul(p2[:, fs], p2[:, fs],
                                                    ky0 / ky2)
                        nc.vector.tensor_scalar_mul(p1[:, fs], p1[:, fs],
                                                    ky1 / ky2)
                    else:
                        nc.scalar.mul(p2[:, fs], p2[:, fs], ky0 / ky2)
                        nc.scalar.mul(p1[:, fs], p1[:, fs], ky1 / ky2)
                    nc.vector.tensor_add(p1[:, fs], p1[:, fs], p2[:, fs])
                    nc.vector.tensor_add(Sv[:, fs], p1[:, fs],
                                         hview(ws, 2, f0, nf))

                # --- CP_m = (inv^m/m!) c^m, precomputed on host,
                #     duplicated per chain: CP[p, m, chain, FO] ---
                CP = res_pool.tile([P, M, 2, FO], FP16, tag="cp")
                nc.sync.dma_start(
                    out=CP[:, :, :, :],
                    in_=xcp[g, :, :].rearrange("p (m c f) -> p m c f",
                                               m=M, c=2))

                # --- num/den series, both chains packed per op:
                #   acc[:, chain*FO+f]: chain 0 -> den (fields 0..M),
                #   chain 1 -> num (fields 1..M+1) ---
                sb = S[:, :]
                T = res_pool.tile([P, M, 2, FO], FP16, tag="T")
                svm = bass.AP(tensor=sb.tensor, offset=sb.offset + FO,
                              ap=[list(sb.ap[0]), [FO, M], [FO, 2], [1, FO]])
                nc.vector.tensor_mul(T[:, :, :, :], CP[:, :, :, :], svm)
                acc = res_pool.tile([P, 2 * FO], FP16, tag="acc")
                nc.vector.tensor_add(acc[:, :], S[:, 0:2 * FO],
                                     T[:, 0, :, :].rearrange("p c f -> p (c f)"))
                for m in range(1, M):
                    nc.vector.tensor_add(
                        acc[:, :], acc[:, :],
                        T[:, m, :, :].rearrange("p c f -> p (c f)"))
                den = acc[:, 0:FO]
                num = acc[:, FO:2 * FO]

                # --- out = num/den (fp32); the last group's cast runs
                # on DVE to keep the kernel tail on one engine ---
                accf = res_pool.tile([P, 2 * FO], FP32, tag="accf")
                if g == NG - 1:
                    nc.vector.tensor_copy(accf[:, 0:FO], acc[:, 0:FO])
                    nc.vector.tensor_copy(accf[:, FO:2 * FO],
                                          acc[:, FO:2 * FO])
                else:
                    nc.scalar.activation(accf[:, :], acc[:, :], AF.Copy)
                rec = res_pool.tile([P, FO], FP32, tag="rec")
                nc.vector.reciprocal_approx_fast(rec[:, :], accf[:, 0:FO])
                r = res_pool.tile([P, FO], FP32, tag="r")
                nc.vector.tensor_mul(r[:, :], rec[:, :], accf[:, FO:2 * FO])
                nc.sync.dma_start(out=out[g, :, :], in_=r[:, :])
    nc.finalize()
    return nc


_NC_CACHE: dict = {}


def _get_nc(gw: np.ndarray) -> bass.Bass:
    key = gw.tobytes()
    if key not in _NC_CACHE:
        _NC_CACHE[key] = _build_nc(gw)
    return _NC_CACHE[key]


def _host_prep(x: np.ndarray, gw: np.ndarray):
    """Shard + relayout on host. Returns in_maps for the 8 cores."""
    xp = np.pad(x, ((0, 0), (0, 0), (PAD, PAD), (PAD, PAD)), mode="edge")
    xp = xp.reshape(CH, H + 2 * PAD, W + 2 * PAD)          # [12, 516, 516]
    xp16 = xp.astype(np.float16)

    gw64 = np.asarray(gw, np.float64)
    gwx = gw64.sum(axis=0)   # W-direction 1D kernel (shift j)
    b1 = np.zeros((P, P), np.float16)
    b2 = np.zeros((4, P), np.float16)
    for mcol in range(P):
        for j in range(K):
            k = mcol + j
            if k < P:
                b1[k, mcol] = gwx[j]
            else:
                b2[k - P, mcol] = gwx[j]

    # fields G_m = f(x) * x^m over the whole padded image, fp16
    inv = 1.0 / (SIGMA * SIGMA)
    x32 = xp16.astype(np.float32)
    fx = np.exp(-x32 * x32 * (inv / 2.0))
    F = np.empty((NF, CH, H + 2 * PAD, W + 2 * PAD), np.float16)
    fm = fx
    F[0] = fm.astype(np.float16)
    for m in range(1, NF):
        fm = fm * x32
        F[m] = fm.astype(np.float16)

    in_maps = []
    for core in range(NCORES):
        r0 = core * RPC
        strip = xp16[:, r0:r0 + SR, :]                     # [12, 68, 516]
        fstr = F[:, :, r0:r0 + SR, :]                      # [NF, 12, 68, 516]
        fswt = fstr.transpose(3, 0, 2, 1)                  # [516, NF, 68, 12]
        gfv = np.ascontiguousarray(
            fswt[:W].reshape(NG, P, NF * FI))              # [4, 128, NF*816]
        gev = np.ascontiguousarray(
            fswt[W:].reshape(4, NF * FI))                  # [4, NF*816]
        ctr = strip[:, PAD:PAD + RPC, PAD:PAD + W]         # [12, 64, 512]
        ctr_t = ctr.transpose(2, 1, 0).astype(np.float32)  # [512, 64, 12]
        cps = []
        cp = np.ones_like(ctr_t)
        for m in range(1, M + 1):
            cp = cp * ctr_t * (inv / m)
            cps.append(cp.astype(np.float16))
        cpstack = np.stack(cps, axis=1)                    # [512, M, 64, 12]
        cpdup = np.repeat(cpstack[:, :, None], 2, axis=2)  # [512, M, 2, 64, 12]
        xcpv = np.ascontiguousarray(
            cpdup.reshape(NG, P, M * 2 * FO))              # [4, 128, M*2*768]
        in_maps.append({"gf": gfv, "ge": gev, "xcp": xcpv, "b1": b1,
                       "b2": b2})
    return in_maps


def run(x: np.ndarray, gw: np.ndarray, trace: bool = False):
    x = np.asarray(x, np.float32)
    gw = np.asarray(gw, np.float32)
    assert x.shape == (B, C, H, W) and gw.shape == (K, K)

    in_maps = _host_prep(x, gw)
    nc = _get_nc(gw)
    res = run_bass_kernel_spmd(nc, in_maps, list(range(NCORES)), trace=trace)

    full = np.empty((B, C, H, W), np.float32)
    for core in range(NCORES):
        o = res.results[core]["out"].reshape(W, RPC, CH)   # [512, 64, 12]
        o = o.transpose(2, 1, 0).reshape(B, C, RPC, W)
        full[:, :, core * RPC:(core + 1) * RPC, :] = o
    return full, res


def kernel(**inputs) -> np.ndarray:
    out, _ = run(inputs["x"], inputs["gw"])
    return out



# revision 6
# speedup vs baseline: 1.6434x; 1.6434x over previous
"""Bilateral filter (5x5, sigma_space = sigma_density = 1.1) on 8 TRN2 NeuronCores.

Contract: kernel(x, gw) takes FULL inputs
    x : [4, 3, 512, 512] float32
    gw: [5, 5] float32 (normalized spatial gaussian)
returns FULL output [4, 3, 512, 512] float32.

Sharding: pure data parallel over H. Core k owns output rows [64k, 64k+64)
of every (b, c) channel; the host hands it an edge-padded strip, so the
device kernel needs no boundary handling and no inter-core communication.

Device algorithm: rank-3 separable expansion of the range kernel with
ratio-aware least-squares coefficients. With inv = 1/sigma^2 and
f(u) = exp(-u^2*inv/2):
    exp(-(p-c)^2*inv/2) = f(p) * f(c) * exp(p*c*inv)
f(c) cancels in the num/den ratio, and exp(p*c*inv) is approximated as
    den ~ d0 + d1*c*p + d2*c^2*p^2          (on the f(p)*p^m field basis)
    num ~ n0*p + n1*c*p^2 + n2*c^2*p^2
where (d, n) are fit jointly to minimize the error of the RATIO num/den
(errors of the two chains correlate and cancel), giving ~6e-3 rel err
with only 3 convolved fields G_m = f(x)*x^m, m = 0..2.

Layout: W(columns) on SBUF partitions (4 groups of 128), free dim is
[row][channel]. The whole separable 5x5 conv runs on the TensorEngine:
the W-direction is a banded-matrix matmul, and the H-direction taps are
folded into 5 PSUM-accumulated matmuls whose lhsT is the banded matrix
scaled by each H tap, reading the rhs at 5 row-shifted free offsets.
The 4 halo columns (next group) contribute via one extra matmul with a
20-partition lhsT (5 shifts x 4 edge cols merged). Fields are computed
on device (ScalarE square/exp + DVE/GpSimd muls) from the raw fp16 x
strip, so HBM traffic is ~2.6MB/core instead of ~10MB. The series is a
packed 2-chain Horner in c on DVE; division is reciprocal_approx_fast.
"""

import numpy as np

import concourse.bass as bass
import concourse.bacc as bacc
import concourse.tile as tile
from concourse import mybir
from concourse.bass_utils import run_bass_kernel_spmd

# ---- problem constants (hardcoded per contract) ----
B, C, H, W = 4, 3, 512, 512
K = 5
PAD = 2
SIGMA = 0.3 * ((K - 1) * 0.5 - 1) + 0.8  # 1.1
INV = 1.0 / (SIGMA * SIGMA)
NCORES = 8
CH = B * C                    # 12 channels
RPC = H // NCORES             # 64 output rows per core
SR = RPC + 2 * PAD            # 68 input rows per channel strip
P = 128
NG = W // P                   # 4 column groups
FI = SR * CH                  # 816 free elems of input-row fields [row][ch]
FO = RPC * CH                 # 768 free elems of output-row tensors [row][ch]
NF = 3                        # fields G_0..G_2

FP32 = mybir.dt.float32
FP16 = mybir.dt.float16
AL = mybir.AluOpType
AF = mybir.ActivationFunctionType


def _fit_coefs():
    """Ratio-aware LS fit of exp(inv*p*c) on the sparse supports
    den {(0,0),(1,1),(2,2)}, num {(0,1),(1,2),(2,2)} (c^k * p^m)."""
    npts = 160
    p = np.linspace(0, 1, npts)
    c = np.linspace(0, 1, npts)
    Pg, Cg = np.meshgrid(p, c, indexing="ij")
    E = np.exp(INV * Pg * Cg)
    w = np.exp(-Pg ** 2 * INV / 2) ** 2
    alpha = 0.3
    bd = [np.ones_like(Pg), Cg * Pg, (Cg * Pg) ** 2]
    bn = [Pg, Cg * Pg ** 2, (Cg * Pg) ** 2]
    A1 = np.concatenate(
        [np.stack([(-Pg * b * w).ravel() for b in bd], 1),
         np.stack([(b * w).ravel() for b in bn], 1)], axis=1)
    A2 = np.concatenate(
        [np.stack([(b * w * alpha).ravel() for b in bd], 1),
         np.zeros((A1.shape[0], 3))], axis=1)
    A = np.concatenate([A1, A2], 0)
    y = np.concatenate([np.zeros(A1.shape[0]), (E * w * alpha).ravel()], 0)
    sol = np.linalg.lstsq(A, y, rcond=None)[0]
    d0, d1, d2, n0, n1, n2 = sol
    return {
        "cd": d1 / d0, "kd": d2 * d0 / d1 ** 2,
        "cn": n1 / n0, "kn": n2 * n0 / n1 ** 2,
        "osc": n0 / d0,
    }


_COEFS = _fit_coefs()


def _build_nc(gw: np.ndarray) -> bass.Bass:
    cf = _COEFS
    nc = bacc.Bacc(None)
    b1d = nc.declare_dram_parameter("b1s", [P, 5 * P], FP16, isOutput=False)
    b2d = nc.declare_dram_parameter("b2m", [4 * K, P], FP16, isOutput=False)
    xsd = nc.declare_dram_parameter("xs", [NG, P, FI], FP16, isOutput=False)
    xed = nc.declare_dram_parameter("xe", [4, FI], FP16, isOutput=False)
    csd = nc.declare_dram_parameter("cs", [NG, P, FO], FP16, isOutput=False)
    outd = nc.declare_dram_parameter("out", [NG, P, FO], FP16, isOutput=True)

    with tile.TileContext(nc) as tc:
        with (
            tc.tile_pool(name="const", bufs=1) as cpool,
            tc.tile_pool(name="flds", bufs=1) as fpool,
            tc.tile_pool(name="sq", bufs=2) as sqpool,
            tc.tile_pool(name="ps", bufs=4, space="PSUM") as pspool,
            tc.tile_pool(name="ser", bufs=2) as spool,
        ):
            b1t = cpool.tile([P, 5 * P], FP16, tag="b1s")
            nc.sync.dma_start(out=b1t[:, :], in_=b1d[:, :])
            b2t = cpool.tile([4 * K, P], FP16, tag="b2m")
            nc.sync.dma_start(out=b2t[:, :], in_=b2d[:, :])

            xs = []
            cs = []
            for g in range(NG):
                xt = cpool.tile([P, FI], FP16, tag=f"xs{g}")
                nc.sync.dma_start(out=xt[:, :], in_=xsd[g, :, :])
                xs.append(xt)
                ct = cpool.tile([P, FO], FP16, tag=f"cs{g}")
                nc.sync.dma_start(out=ct[:, :], in_=csd[g, :, :])
                cs.append(ct)
            xe = cpool.tile([4, FI], FP16, tag="xe")
            nc.sync.dma_start(out=xe[:, :], in_=xed[:, :])

            # --- fields G_m = f(x) x^m, on device ---
            # G layout per group: [P, NF*FI], field m at offset m*FI
            G = []
            for g in range(NG):
                G.append(fpool.tile([P, NF * FI], FP16, tag=f"g{g}",
                                    name=f"gfld{g}"))
            Ge = fpool.tile([4, NF * FI], FP16, tag="ge")

            for g in range(NG + 1):
                src = xs[g] if g < NG else xe
                dst = G[g] if g < NG else Ge
                pp = P if g < NG else 4
                sq = sqpool.tile([P, FI], FP16, tag="sq")
                nc.scalar.square(sq[0:pp, :], src[:, :])
                nc.scalar.activation(dst[0:pp, 0:FI], sq[0:pp, :], AF.Exp,
                                     scale=-INV / 2.0)
                nc.vector.tensor_mul(dst[0:pp, FI:2 * FI], dst[0:pp, 0:FI],
                                     src[:, :])
                nc.gpsimd.tensor_mul(dst[0:pp, 2 * FI:3 * FI],
                                     dst[0:pp, FI:2 * FI], src[:, :])

            # --- halo tiles: 5 row-shifted copies of the next group's
            # first 4 columns, on 20 partitions ---
            halo = []
            for g in range(NG):
                ht = fpool.tile([4 * K, NF * FO], FP16, tag=f"h{g}",
                                name=f"halo{g}")
                nbr = G[g + 1] if g + 1 < NG else Ge
                for i in range(K):
                    base = nbr[0:4, :]
                    src_v = bass.AP(tensor=base.tensor,
                                    offset=base.offset + i * CH,
                                    ap=[list(base.ap[0]), [FI, NF], [1, FO]])
                    nc.sync.dma_start(out=ht[4 * i:4 * i + 4, :], in_=src_v)
                halo.append(ht)

            for g in range(NG):
                # --- full separable 5x5 conv on TensorE: PSUM accumulates
                # 5 H-shifted banded-W matmuls + 1 merged halo matmul ---
                ps = [pspool.tile([P, FO], FP32, tag="ps", name=f"ps{g}_{m}")
                      for m in range(NF)]
                chunks = ((0, 512), (512, FO - 512))
                for i in range(K):
                    lhs = b1t[:, i * P:(i + 1) * P]
                    for m in range(NF):
                        for o, sz in chunks:
                            rhs = G[g][:, m * FI + i * CH + o:
                                       m * FI + i * CH + o + sz]
                            nc.tensor.matmul(ps[m][:, o:o + sz], lhs, rhs,
                                             start=(i == 0), stop=False)
                for m in range(NF):
                    for o, sz in chunks:
                        nc.tensor.matmul(ps[m][:, o:o + sz], b2t[:, :],
                                         halo[g][:, m * FO + o:m * FO + o + sz],
                                         start=False, stop=True)

                # --- PSUM -> SBUF: S stack [S0|S1|S2] fp16 (ScalarE) ---
                St = spool.tile([P, NF * FO], FP16, tag="St")
                for m in range(NF):
                    nc.scalar.activation(St[:, m * FO:(m + 1) * FO],
                                         ps[m][:, :], AF.Copy)

                # --- packed 2-chain Horner in c (DVE) ---
                # den/d0 = S0 + c'(S1 + c'*kd*S2),  c' = (d1/d0) c
                # num/n0 = S1 + c''(S2 + c''*kn*S2), c'' = (n1/n0) c
                cp = spool.tile([P, 2 * FO], FP16, tag="cp")
                nc.vector.tensor_scalar_mul(cp[:, 0:FO], cs[g][:, :],
                                            float(cf["cd"]))
                nc.vector.tensor_scalar_mul(cp[:, FO:2 * FO], cs[g][:, :],
                                            float(cf["cn"]))
                L2 = spool.tile([P, 2 * FO], FP16, tag="L2")
                nc.vector.tensor_scalar_mul(L2[:, 0:FO],
                                            St[:, 2 * FO:3 * FO],
                                            float(cf["kd"]))
                nc.vector.tensor_scalar_mul(L2[:, FO:2 * FO],
                                            St[:, 2 * FO:3 * FO],
                                            float(cf["kn"]))
                acc = spool.tile([P, 2 * FO], FP16, tag="acc")
                nc.vector.tensor_mul(acc[:, :], cp[:, :], L2[:, :])
                nc.vector.tensor_add(acc[:, :], acc[:, :],
                                     St[:, FO:3 * FO])
                nc.vector.tensor_mul(acc[:, :], acc[:, :], cp[:, :])
                nc.vector.tensor_add(acc[:, :], acc[:, :], St[:, 0:2 * FO])

                # --- out = num * recip(den/osc),  osc = n0/d0 folded into
                # the den evacuation scale ---
                denf = spool.tile([P, FO], FP32, tag="denf")
                nc.scalar.mul(denf[:, :], acc[:, 0:FO], 1.0 / float(cf["osc"]))
                rec = spool.tile([P, FO], FP32, tag="rec")
                nc.vector.reciprocal_approx_fast(rec[:, :], denf[:, :])
                o = spool.tile([P, FO], FP16, tag="o")
                nc.gpsimd.tensor_mul(o[:, :], acc[:, FO:2 * FO], rec[:, :])
                nc.sync.dma_start(out=outd[g, :, :], in_=o[:, :])
    nc.finalize()
    return nc


_NC_CACHE: dict = {}


def _get_nc(gw: np.ndarray) -> bass.Bass:
    key = gw.tobytes()
    if key not in _NC_CACHE:
        _NC_CACHE[key] = _build_nc(gw)
    return _NC_CACHE[key]


def _host_prep(x: np.ndarray, gw: np.ndarray):
    """Shard + relayout on host. Returns in_maps for the 8 cores."""
    gw64 = np.asarray(gw, np.float64)
    gwy = gw64.sum(axis=1)   # H-direction taps (row shift i)
    gwx = gw64.sum(axis=0)   # W-direction taps (col shift j)

    b1s = np.zeros((P, 5 * P), np.float16)
    for i in range(K):
        for mcol in range(P):
            for j in range(K):
                k = mcol + j
                if k < P:
                    b1s[k, i * P + mcol] = gwy[i] * gwx[j]
    b2m = np.zeros((4 * K, P), np.float16)
    for i in range(K):
        for e in range(4):
            for mcol in range(P - 4, P):
                j = 128 + e - mcol
                if 0 <= j < K:
                    b2m[4 * i + e, mcol] = gwy[i] * gwx[j]

    xp = np.pad(x, ((0, 0), (0, 0), (PAD, PAD), (PAD, PAD)), mode="edge")
    xp16 = xp.reshape(CH, H + 2 * PAD, W + 2 * PAD).astype(np.float16)
    x16 = x.reshape(CH, H, W).astype(np.float16)

    in_maps = []
    for core in range(NCORES):
        r0 = core * RPC
        strip = xp16[:, r0:r0 + SR, :]                 # [12, 68, 516]
        st = np.ascontiguousarray(strip.transpose(2, 1, 0))  # [516, 68, 12]
        xs = st[:W].reshape(NG, P, FI)
        xe = st[W:].reshape(4, FI)
        ctr = x16[:, r0:r0 + RPC, :]                   # [12, 64, 512]
        ct = np.ascontiguousarray(ctr.transpose(2, 1, 0))    # [512, 64, 12]
        csv = ct.reshape(NG, P, FO)
        in_maps.append({"b1s": b1s, "b2m": b2m, "xs": xs, "xe": xe,
                        "cs": csv})
    return in_maps


def run(x: np.ndarray, gw: np.ndarray, trace: bool = False):
    x = np.asarray(x, np.float32)
    gw = np.asarray(gw, np.float32)
    assert x.shape == (B, C, H, W) and gw.shape == (K, K)

    in_maps = _host_prep(x, gw)
    nc = _get_nc(gw)
    res = run_bass_kernel_spmd(nc, in_maps, list(range(NCORES)), trace=trace)

    full = np.empty((B, C, H, W), np.float32)
    for core in range(NCORES):
        o = res.results[core]["out"].astype(np.float32)
        o = o.reshape(NG, P, RPC, CH).transpose(3, 2, 0, 1)
        full[:, :, core * RPC:(core + 1) * RPC, :] = o.reshape(
            B, C, RPC, W)
    return full, res


def kernel(**inputs) -> np.ndarray:
    out, _ = run(inputs["x"], inputs["gw"])
    return out


# revision 9
# speedup vs baseline: 1.6473x; 1.0024x over previous
"""Bilateral filter (5x5, sigma_space = sigma_density = 1.1) on 8 TRN2 NeuronCores.

Contract: kernel(x, gw) takes FULL inputs
    x : [4, 3, 512, 512] float32
    gw: [5, 5] float32 (normalized spatial gaussian)
returns FULL output [4, 3, 512, 512] float32.

Sharding: pure data parallel over H. Core k owns output rows [64k, 64k+64)
of every (b, c) channel; the host hands it an edge-padded strip, so the
device kernel needs no boundary handling and no inter-core communication.

Device algorithm: rank-3 separable expansion of the range kernel with
ratio-aware least-squares coefficients. With inv = 1/sigma^2 and
f(u) = exp(-u^2*inv/2):
    exp(-(p-c)^2*inv/2) = f(p) * f(c) * exp(p*c*inv)
f(c) cancels in the num/den ratio, and exp(p*c*inv) is approximated as
    den ~ d0 + d1*c*p + d2*c^2*p^2          (on the f(p)*p^m field basis)
    num ~ n0*p + n1*c*p^2 + n2*c^2*p^2
where (d, n) are fit jointly to minimize the error of the RATIO num/den
(errors of the two chains correlate and cancel), giving ~6e-3 rel err
with only 3 convolved fields G_m = f(x)*x^m, m = 0..2.

Layout: W(columns) on SBUF partitions (4 groups of 128), free dim is
[row][channel]. The whole separable 5x5 conv runs on the TensorEngine:
the W-direction is a banded-matrix matmul, and the H-direction taps are
folded into 5 PSUM-accumulated matmuls whose lhsT is the banded matrix
scaled by each H tap, reading the rhs at 5 row-shifted free offsets.
The 4 halo columns (next group) contribute via one extra matmul with a
20-partition lhsT (5 shifts x 4 edge cols merged). Fields are computed
on device (ScalarE square/exp + DVE/GpSimd muls) from the raw fp16 x
strip, so HBM traffic is ~2.6MB/core instead of ~10MB. The series is a
packed 2-chain Horner in c on DVE; division is reciprocal_approx_fast.
"""

import numpy as np

import concourse.bass as bass
import concourse.bacc as bacc
import concourse.tile as tile
from concourse import mybir
from concourse.bass_utils import run_bass_kernel_spmd

# ---- problem constants (hardcoded per contract) ----
B, C, H, W = 4, 3, 512, 512
K = 5
PAD = 2
SIGMA = 0.3 * ((K - 1) * 0.5 - 1) + 0.8  # 1.1
INV = 1.0 / (SIGMA * SIGMA)
NCORES = 8
CH = B * C                    # 12 channels
RPC = H // NCORES             # 64 output rows per core
SR = RPC + 2 * PAD            # 68 input rows per channel strip
P = 128
NG = W // P                   # 4 column groups
FI = SR * CH                  # 816 free elems of input-row fields [row][ch]
FO = RPC * CH                 # 768 free elems of output-row tensors [row][ch]
NF = 3                        # fields G_0..G_2

FP32 = mybir.dt.float32
FP16 = mybir.dt.float16
AL = mybir.AluOpType
AF = mybir.ActivationFunctionType


def _fit_coefs():
    """Ratio-aware LS fit of exp(inv*p*c) on the sparse supports
    den {(0,0),(1,1),(2,2)}, num {(0,1),(1,2),(2,2)} (c^k * p^m)."""
    npts = 160
    p = np.linspace(0, 1, npts)
    c = np.linspace(0, 1, npts)
    Pg, Cg = np.meshgrid(p, c, indexing="ij")
    E = np.exp(INV * Pg * Cg)
    w = np.exp(-Pg ** 2 * INV / 2) ** 2
    alpha = 0.3
    bd = [np.ones_like(Pg), Cg * Pg, (Cg * Pg) ** 2]
    bn = [Pg, Cg * Pg ** 2, (Cg * Pg) ** 2]
    A1 = np.concatenate(
        [np.stack([(-Pg * b * w).ravel() for b in bd], 1),
         np.stack([(b * w).ravel() for b in bn], 1)], axis=1)
    A2 = np.concatenate(
        [np.stack([(b * w * alpha).ravel() for b in bd], 1),
         np.zeros((A1.shape[0], 3))], axis=1)
    A = np.concatenate([A1, A2], 0)
    y = np.concatenate([np.zeros(A1.shape[0]), (E * w * alpha).ravel()], 0)
    sol = np.linalg.lstsq(A, y, rcond=None)[0]
    d0, d1, d2, n0, n1, n2 = sol
    return {
        "cd": d1 / d0, "kd": d2 * d0 / d1 ** 2,
        "cn": n1 / n0, "kn": n2 * n0 / n1 ** 2,
        "osc": n0 / d0,
    }


_COEFS = _fit_coefs()


def _build_nc(gw: np.ndarray) -> bass.Bass:
    cf = _COEFS
    nc = bacc.Bacc(None)
    b1d = nc.declare_dram_parameter("b1s", [P, 5 * P], FP16, isOutput=False)
    b2d = nc.declare_dram_parameter("b2m", [4 * K, P], FP16, isOutput=False)
    xsd = nc.declare_dram_parameter("xs", [NG, P, FI], FP16, isOutput=False)
    xed = nc.declare_dram_parameter("xe", [4, FI], FP16, isOutput=False)
    csd = nc.declare_dram_parameter("cs", [NG, P, FO], FP16, isOutput=False)
    outd = nc.declare_dram_parameter("out", [NG, P, FO], FP16, isOutput=True)

    with tile.TileContext(nc) as tc:
        with (
            tc.tile_pool(name="const", bufs=1) as cpool,
            tc.tile_pool(name="flds", bufs=1) as fpool,
            tc.tile_pool(name="sq", bufs=2) as sqpool,
            tc.tile_pool(name="ps", bufs=4, space="PSUM") as pspool,
            tc.tile_pool(name="ser", bufs=2) as spool,
        ):
            b1t = cpool.tile([P, 5 * P], FP16, tag="b1s")
            nc.sync.dma_start(out=b1t[:, :], in_=b1d[:, :])
            b2t = cpool.tile([4 * K, P], FP16, tag="b2m")
            nc.sync.dma_start(out=b2t[:, :], in_=b2d[:, :])

            xs = []
            cs = []
            for g in range(NG):
                xt = cpool.tile([P, FI], FP16, tag=f"xs{g}")
                nc.sync.dma_start(out=xt[:, :], in_=xsd[g, :, :])
                xs.append(xt)
                ct = cpool.tile([P, FO], FP16, tag=f"cs{g}")
                nc.sync.dma_start(out=ct[:, :], in_=csd[g, :, :])
                cs.append(ct)
            xe = cpool.tile([4, FI], FP16, tag="xe")
            nc.sync.dma_start(out=xe[:, :], in_=xed[:, :])

            # --- fields G_m = f(x) x^m, on device ---
            # G layout per group: [P, NF*FI], field m at offset m*FI
            G = []
            for g in range(NG):
                G.append(fpool.tile([P, NF * FI], FP16, tag=f"g{g}",
                                    name=f"gfld{g}"))
            Ge = fpool.tile([4, NF * FI], FP16, tag="ge")

            for g in range(NG + 1):
                src = xs[g] if g < NG else xe
                dst = G[g] if g < NG else Ge
                pp = P if g < NG else 4
                sq = sqpool.tile([P, FI], FP16, tag="sq")
                # group 0 is on the kernel-startup critical path: keep its
                # field chain off the slower engines
                if g == 0:
                    nc.vector.tensor_mul(sq[0:pp, :], src[:, :], src[:, :])
                else:
                    nc.scalar.square(sq[0:pp, :], src[:, :])
                nc.scalar.activation(dst[0:pp, 0:FI], sq[0:pp, :], AF.Exp,
                                     scale=-INV / 2.0)
                nc.vector.tensor_mul(dst[0:pp, FI:2 * FI], dst[0:pp, 0:FI],
                                     src[:, :])
                if g <= 1:
                    nc.vector.tensor_mul(dst[0:pp, 2 * FI:3 * FI],
                                         dst[0:pp, FI:2 * FI], src[:, :])
                else:
                    nc.gpsimd.tensor_mul(dst[0:pp, 2 * FI:3 * FI],
                                         dst[0:pp, FI:2 * FI], src[:, :])

            # --- halo tiles: 5 row-shifted copies of the next group's
            # first 4 columns, on 20 partitions ---
            halo = []
            for g in range(NG):
                ht = fpool.tile([4 * K, NF * FO], FP16, tag=f"h{g}",
                                name=f"halo{g}")
                nbr = G[g + 1] if g + 1 < NG else Ge
                for i in range(K):
                    base = nbr[0:4, :]
                    src_v = bass.AP(tensor=base.tensor,
                                    offset=base.offset + i * CH,
                                    ap=[list(base.ap[0]), [FI, NF], [1, FO]])
                    nc.sync.dma_start(out=ht[4 * i:4 * i + 4, :], in_=src_v)
                halo.append(ht)

            # --- packed per-chain normalized c: cp = [c'|c''] per group
            # (cheap DVE 4x ops; emitted early, executed in DVE idle time) ---
            cps = []
            for g in range(NG):
                cp = spool.tile([P, 2 * FO], FP16, tag=f"cp{g}",
                                name=f"cp{g}")
                nc.vector.tensor_scalar_mul(cp[:, 0:FO], cs[g][:, :],
                                            float(cf["cd"]))
                nc.vector.tensor_scalar_mul(cp[:, FO:2 * FO], cs[g][:, :],
                                            float(cf["cn"]))
                cps.append(cp)

            HFO = FO // 2   # series processed in two half-row chunks

            for g in range(NG):
                # --- full separable 5x5 conv on TensorE: PSUM accumulates
                # 5 H-shifted banded-W matmuls + 1 merged halo matmul.
                # Field-outer order keeps 6 consecutive matmuls on one PSUM
                # tile (avoids PE throttle oscillation from bank cycling);
                # one N=768 matmul per (field, shift) - fp16 moving operand
                # allows up to 1024. ---
                ps = [pspool.tile([P, FO], FP32, tag="ps", name=f"ps{g}_{m}")
                      for m in range(NF)]
                chunks = ((0, 512), (512, FO - 512))
                for m in range(NF):
                    for i in range(K):
                        base = m * FI + i * CH
                        for o, sz in chunks:
                            nc.tensor.matmul(ps[m][:, o:o + sz],
                                             b1t[:, i * P:(i + 1) * P],
                                             G[g][:, base + o:base + o + sz],
                                             start=(i == 0), stop=False)
                    for o, sz in chunks:
                        nc.tensor.matmul(ps[m][:, o:o + sz], b2t[:, :],
                                         halo[g][:, m * FO + o:m * FO + o + sz],
                                         start=False, stop=True)

                # --- PSUM -> SBUF S stack [S0|S1|S2] fp16 (ScalarE), in two
                # half chunks, S2 first (unblocks the Horner chain) ---
                St = spool.tile([P, NF * FO], FP16, tag="St")
                for o in (0, HFO):
                    for m in (2, 1, 0):
                        nc.scalar.activation(
                            St[:, m * FO + o:m * FO + o + HFO],
                            ps[m][:, o:o + HFO], AF.Copy)

                for o in (0, HFO):
                    last = g == NG - 1 and o == HFO
                    # packed [den|num] views of cp / St for this chunk
                    def pview(t, off):
                        b = t[:, :]
                        return bass.AP(tensor=b.tensor, offset=b.offset + off,
                                       ap=[list(b.ap[0]), [FO, 2], [1, HFO]])
                    cpv = pview(cps[g], o)
                    # --- packed 2-chain Horner in c (DVE) ---
                    # den/d0 = S0 + c'(S1 + c'*kd*S2),  c' = (d1/d0) c
                    # num/n0 = S1 + c''(S2 + c''*kn*S2), c'' = (n1/n0) c
                    L2 = spool.tile([P, 2 * HFO], FP16, tag="L2")
                    nc.vector.tensor_scalar_mul(
                        L2[:, 0:HFO], St[:, 2 * FO + o:2 * FO + o + HFO],
                        float(cf["kd"]))
                    nc.vector.tensor_scalar_mul(
                        L2[:, HFO:2 * HFO], St[:, 2 * FO + o:2 * FO + o + HFO],
                        float(cf["kn"]))
                    acc = spool.tile([P, 2 * HFO], FP16, tag="acc")
                    nc.vector.tensor_mul(acc[:, :], cpv, L2[:, :])
                    nc.vector.tensor_add(acc[:, :], acc[:, :],
                                         pview(St, FO + o))
                    nc.vector.tensor_mul(acc[:, :], acc[:, :], cpv)
                    nc.vector.tensor_add(acc[:, :], acc[:, :], pview(St, o))

                    # --- out = num * recip(den/osc); osc folded into the
                    # den->fp32 copy scale. Last chunk runs its whole finale
                    # on DVE (shortest kernel tail); others spread across
                    # ScalarE/GpSimd ---
                    denf = spool.tile([P, HFO], FP32, tag="denf")
                    if last:
                        nc.vector.tensor_scalar_mul(denf[:, :],
                                                    acc[:, 0:HFO],
                                                    1.0 / float(cf["osc"]))
                    else:
                        nc.scalar.mul(denf[:, :], acc[:, 0:HFO],
                                      1.0 / float(cf["osc"]))
                    rec = spool.tile([P, HFO], FP32, tag="rec")
                    nc.vector.reciprocal_approx_fast(rec[:, :], denf[:, :])
                    o_t = spool.tile([P, HFO], FP16, tag="o")
                    if last:
                        nc.vector.tensor_mul(o_t[:, :], acc[:, HFO:2 * HFO],
                                             rec[:, :])
                    else:
                        nc.gpsimd.tensor_mul(o_t[:, :], acc[:, HFO:2 * HFO],
                                             rec[:, :])
                    nc.sync.dma_start(out=outd[g, :, o:o + HFO],
                                      in_=o_t[:, :])
    nc.finalize()
    return nc


_NC_CACHE: dict = {}


def _get_nc(gw: np.ndarray) -> bass.Bass:
    key = gw.tobytes()
    if key not in _NC_CACHE:
        _NC_CACHE[key] = _build_nc(gw)
    return _NC_CACHE[key]


def _host_prep(x: np.ndarray, gw: np.ndarray):
    """Shard + relayout on host. Returns in_maps for the 8 cores."""
    gw64 = np.asarray(gw, np.float64)
    gwy = gw64.sum(axis=1)   # H-direction taps (row shift i)
    gwx = gw64.sum(axis=0)   # W-direction taps (col shift j)

    b1s = np.zeros((P, 5 * P), np.float16)
    for i in range(K):
        for mcol in range(P):
            for j in range(K):
                k = mcol + j
                if k < P:
                    b1s[k, i * P + mcol] = gwy[i] * gwx[j]
    b2m = np.zeros((4 * K, P), np.float16)
    for i in range(K):
        for e in range(4):
            for mcol in range(P - 4, P):
                j = 128 + e - mcol
                if 0 <= j < K:
                    b2m[4 * i + e, mcol] = gwy[i] * gwx[j]

    xp = np.pad(x, ((0, 0), (0, 0), (PAD, PAD), (PAD, PAD)), mode="edge")
    xp16 = xp.reshape(CH, H + 2 * PAD, W + 2 * PAD).astype(np.float16)
    x16 = x.reshape(CH, H, W).astype(np.float16)

    in_maps = []
    for core in range(NCORES):
        r0 = core * RPC
        strip = xp16[:, r0:r0 + SR, :]                 # [12, 68, 516]
        st = np.ascontiguousarray(strip.transpose(2, 1, 0))  # [516, 68, 12]
        xs = st[:W].reshape(NG, P, FI)
        xe = st[W:].reshape(4, FI)
        ctr = x16[:, r0:r0 + RPC, :]                   # [12, 64, 512]
        ct = np.ascontiguousarray(ctr.transpose(2, 1, 0))    # [512, 64, 12]
        csv = ct.reshape(NG, P, FO)
        in_maps.append({"b1s": b1s, "b2m": b2m, "xs": xs, "xe": xe,
                        "cs": csv})
    return in_maps


def run(x: np.ndarray, gw: np.ndarray, trace: bool = False):
    x = np.asarray(x, np.float32)
    gw = np.asarray(gw, np.float32)
    assert x.shape == (B, C, H, W) and gw.shape == (K, K)

    in_maps = _host_prep(x, gw)
    nc = _get_nc(gw)
    res = run_bass_kernel_spmd(nc, in_maps, list(range(NCORES)), trace=trace)

    full = np.empty((B, C, H, W), np.float32)
    for core in range(NCORES):
        o = res.results[core]["out"].astype(np.float32)
        o = o.reshape(NG, P, RPC, CH).transpose(3, 2, 0, 1)
        full[:, :, core * RPC:(core + 1) * RPC, :] = o.reshape(
            B, C, RPC, W)
    return full, res


def kernel(**inputs) -> np.ndarray:
    out, _ = run(inputs["x"], inputs["gw"])
    return out
